# revision 21
# baseline (speedup 1.0000x reference)
"""CountVectorizer Trainium2 kernel (v4: vocab-sharded counts matmul,
active-vocab compaction).

Computes out = counts @ W + b  where counts[b, v] = #{s: token_ids[b, s] == v}.

v2 (embedding-bag dma_gather) was SWDGE descriptor-generation bound:
~7.85 ns/gathered-row on the Q7 => ~282 us serial GpSimd (358 us total).
v3+ uses the dense formulation from the sharding hint: the vocab is sharded
across the 8 cores; each core streams its W shard (bf16) and a host-built
counts shard (fp8 e4m3 -- counts are small ints, exact in e4m3) and runs
   out_c[d, b] = sum_v W[v, d] * counts[v, b]
as accumulating PE matmuls: lhsT = W tile [128v, 128d] bf16 stationary,
rhs = counts tile [128v, 512b] fp8 moving, PSUM f32 (512-col halves --
a matmul cannot cross a PSUM bank).  The host sums the 8 partials and adds
the bias in f32, so the only error source is the bf16 W cast (~1.6e-3 rel,
gate 2e-2).

v4 compacts the vocab first: rows of counts that are all-zero across the
batch (P = (1-1/V)^(B*S) ~ 13%) are pruned on the host, and only active
W/counts rows ship.  Both HBM streams and the PE contraction shrink ~13%.

Per-core after compaction: ~2.8 MB W + ~11.2 MB counts (~39 us at
358 GB/s HBM); PE: ~86 tiles x 1024 cols ~ 40 us warm at 2.4 GHz.  DMAs
are chunked (counts first, small leading chunk) and the matmuls chase the
chunks, overlapping the two almost fully.  Both streams and the PE are at
their rooflines simultaneously.

Measured (local harness, core-0 NTFF): 57.4-61 us vs 358 us for the v2
gather baseline (~6.1x).  Head ~10 us (NEFF preamble + chunk-0 flight)
and tail ~6 us (PSUM drain + out DMA + final barrier) are fixed costs.
"""

import sys
import types

import numpy as np
import ml_dtypes

import concourse.bacc as bacc
import concourse.mybir as mybir
import concourse.tile as tile
from concourse.bass_utils import run_bass_kernel_spmd


def _register_ntff_hook():
    """If the image's antenv lacks axon_hooks, run_bass_kernel_spmd crashes
    under BASS_TRACE=1; synthesize the module from trn_boot's ctypes hook
    builder (silent no-op when unavailable)."""
    try:
        import antenv.axon_hooks  # noqa: F401
        return
    except ImportError:
        pass
    try:
        from trn_agent_boot.trn_boot import _ntff_profile_via_ctypes

        hook = _ntff_profile_via_ctypes("/opt/axon/libaxon_pjrt.so")
    except Exception:
        hook = None
    mod = types.ModuleType("antenv.axon_hooks")
    mod.get_axon_ntff_profile_hook = lambda: hook
    mod.set_axon_ntff_profile_hook = lambda h: None
    sys.modules["antenv.axon_hooks"] = mod


_register_ntff_hook()

B, S, V, D = 1024, 200, 100000, 128
N_CORES = 8
P = 128

_CACHE: dict = {}


def _chunk_sizes(G):
    """DMA chunking in g-tiles: tiny first chunk so matmul 0 starts early,
    ~5-tile steady state, small tail so the last matmuls aren't waiting on
    a big transfer."""
    steady = 5
    if G <= 2:
        return [1] * G
    sizes = [1]
    rem = G - 1
    while rem > steady + 2:
        sizes.append(steady)
        rem -= steady
    if rem > 3:
        sizes.append(rem - 3)
        rem = 3
    while rem:
        c = min(2, rem)
        sizes.append(c)
        rem -= c
    assert sum(sizes) == G
    return sizes


def _build_nc(G):
    nc = bacc.Bacc(
        "TRN2",
        target_bir_lowering=False,
        debug=False,
        num_devices=N_CORES,
    )
    f32 = mybir.dt.float32
    bf16 = mybir.dt.bfloat16
    fp8 = mybir.dt.float8e4

    cnt = nc.dram_tensor("cnt", [P, G * B], fp8, kind="ExternalInput")
    wsh = nc.dram_tensor("wsh", [P, G * D], bf16, kind="ExternalInput")
    out_t = nc.dram_tensor("out_t", [P, B], f32, kind="ExternalOutput")

    with tile.TileContext(nc) as tc:
        with (
            tc.tile_pool(name="const", bufs=1) as cpool,
            tc.tile_pool(name="psum", bufs=1, space="PSUM") as ppool,
        ):
            cnt_sb = cpool.tile([P, G * B], fp8)
            w_sb = cpool.tile([P, G * D], bf16)
            out_sb = cpool.tile([P, B], f32)
            warm_sb = cpool.tile([P, 512], bf16)

            # HAM warm-up sized to the idle window between body start and
            # chunk-0 arrival (~3.5 us): the PE cold clock is 1.2 GHz and
            # unthrottles after ~3.4 us of sustained activity, so these
            # dummy matmuls cost nothing and the real ones start warm.
            # (A 14-matmul chain was a net loss when it overran the window.)
            nc.vector.memset(warm_sb[:], 0.0)
            pwarm = ppool.tile([P, 512], f32, tag="pwarm")
            for k in range(8):
                nc.tensor.matmul(
                    pwarm[:],
                    warm_sb[:, 0:128],
                    warm_sb[:],
                    start=(k == 0),
                    stop=(k == 7),
                )

            # chunked input streams, counts/W pairwise interleaved so the
            # g-th matmul's operands land together (Tile adds per-chunk
            # deps).  (Scalar-queue W dispatch was tried and removed: it
            # starved the counts stream.)
            k = 0
            for sz in _chunk_sizes(G):
                hi = k + sz
                nc.sync.dma_start(
                    out=cnt_sb[:, k * B : hi * B], in_=cnt[:, k * B : hi * B]
                )
                nc.sync.dma_start(
                    out=w_sb[:, k * D : hi * D], in_=wsh[:, k * D : hi * D]
                )
                k = hi

            ps0 = ppool.tile([P, 512], f32, tag="ps0")
            ps1 = ppool.tile([P, 512], f32, tag="ps1")
            for g in range(G):
                w_tile = w_sb[:, g * D : (g + 1) * D]
                nc.tensor.matmul(
                    ps0[:],
                    w_tile,
                    cnt_sb[:, g * B : g * B + 512],
                    start=(g == 0),
                    stop=(g == G - 1),
                )
                nc.tensor.matmul(
                    ps1[:],
                    w_tile,
                    cnt_sb[:, g * B + 512 : (g + 1) * B],
                    start=(g == 0),
                    stop=(g == G - 1),
                )

            # drain per half so copy/out overlap the other half's finish
            nc.vector.tensor_copy(out=out_sb[:, 0:512], in_=ps0[:])
            nc.sync.dma_start(out=out_t[:, 0:512], in_=out_sb[:, 0:512])
            nc.vector.tensor_copy(out=out_sb[:, 512:B], in_=ps1[:])
            nc.sync.dma_start(out=out_t[:, 512:B], in_=out_sb[:, 512:B])

    nc.compile()
    return nc


def _get_nc(G=86):
    key = ("nc", G)
    if key not in _CACHE:
        _CACHE[key] = _build_nc(G)
    return _CACHE[key]


def _shard_layout(arr2d, ncols):
    """[G*128, ncols] -> [128, G*ncols] partition-major: out[p, g*ncols+j]
    = arr2d[g*128 + p, j]."""
    g = arr2d.shape[0] // P
    a = arr2d.reshape(g, P, ncols).transpose(1, 0, 2).reshape(P, g * ncols)
    return np.ascontiguousarray(a)


def _in_maps(token_ids, W, b):
    """Returns (in_maps, G)."""
    counts = np.zeros((B, V), dtype=np.int16)
    rows = np.repeat(np.arange(B, dtype=np.int64), S)
    np.add.at(counts, (rows, token_ids.ravel().astype(np.int64)), 1)
    if counts.max() > 16:
        raise ValueError("count > 16 not exact in fp8 e4m3")

    # active-vocab compaction: ship only rows some batch row references
    active = np.flatnonzero(counts.any(axis=0))
    M = active.size
    per = -(-M // N_CORES)          # rows per core
    G = max(1, -(-per // P))        # 128-row tiles per core
    VP = G * P

    Wb = W.astype(ml_dtypes.bfloat16)
    in_maps = []
    for c in range(N_CORES):
        idx = active[c * per : (c + 1) * per]
        csh = np.zeros((VP, B), dtype=ml_dtypes.float8_e4m3)
        csh[: idx.size] = counts[:, idx].T.astype(ml_dtypes.float8_e4m3)
        wshard = np.zeros((VP, D), dtype=ml_dtypes.bfloat16)
        wshard[: idx.size] = Wb[idx]
        in_maps.append(
            {"cnt": _shard_layout(csh, B), "wsh": _shard_layout(wshard, D)}
        )
    return in_maps, G


def _kernel_numpy(token_ids, W, b):
    out = np.tile(b.astype(np.float32), (B, 1))
    for i in range(B):
        out[i] += W[token_ids[i]].sum(axis=0)
    return out.astype(np.float32)


def kernel(token_ids, W, b, **kwargs):
    token_ids = np.ascontiguousarray(np.asarray(token_ids, dtype=np.int32))
    W = np.ascontiguousarray(np.asarray(W, dtype=np.float32))
    b = np.ascontiguousarray(np.asarray(b, dtype=np.float32))
    assert token_ids.shape == (B, S) and W.shape == (V, D) and b.shape == (D,)

    try:
        in_maps, G = _in_maps(token_ids, W, b)
    except ValueError:
        return _kernel_numpy(token_ids, W, b)

    nc = _get_nc(G)
    res = run_bass_kernel_spmd(nc, in_maps, core_ids=list(range(N_CORES)))
    acc = np.zeros((P, B), dtype=np.float32)
    for c in range(N_CORES):
        acc += np.asarray(res.results[c]["out_t"], dtype=np.float32)
    return (acc.T + b[None, :]).astype(np.float32)


# revision 24
# speedup vs baseline: 1.0661x; 1.0661x over previous
"""CountVectorizer Trainium2 kernel (v4: vocab-sharded counts matmul,
active-vocab compaction).

Computes out = counts @ W + b  where counts[b, v] = #{s: token_ids[b, s] == v}.

v2 (embedding-bag dma_gather) was SWDGE descriptor-generation bound:
~7.85 ns/gathered-row on the Q7 => ~282 us serial GpSimd (358 us total).
v3+ uses the dense formulation from the sharding hint: the vocab is sharded
across the 8 cores; each core streams its W shard (bf16) and a host-built
counts shard (fp8 e4m3 -- counts are small ints, exact in e4m3) and runs
   out_c[d, b] = sum_v W[v, d] * counts[v, b]
as accumulating PE matmuls: lhsT = W tile [128v, 128d] bf16 stationary,
rhs = counts tile [128v, 512b] fp8 moving, PSUM f32 (512-col halves --
a matmul cannot cross a PSUM bank).  The host sums the 8 partials and adds
the bias in f32, so the only error source is the bf16 W cast (~1.6e-3 rel,
gate 2e-2).

v4 compacts the vocab first: rows of counts that are all-zero across the
batch (P = (1-1/V)^(B*S) ~ 13%) are pruned on the host, and only active
W/counts rows ship.  Both HBM streams and the PE contraction shrink ~13%.

Per-core after compaction: ~2.8 MB W + ~11.2 MB counts (~39 us at
358 GB/s HBM); PE: ~86 tiles x 1024 cols ~ 40 us warm at 2.4 GHz.  DMAs
are chunked (counts first, small leading chunk) and the matmuls chase the
chunks, overlapping the two almost fully.  Both streams and the PE are at
their rooflines simultaneously.

Measured (local harness, core-0 NTFF): 57.4-61 us vs 358 us for the v2
gather baseline (~6.1x).  Head ~10 us (NEFF preamble + chunk-0 flight)
and tail ~6 us (PSUM drain + out DMA + final barrier) are fixed costs.
"""

import sys
import types

import numpy as np
import ml_dtypes

import concourse.bacc as bacc
import concourse.mybir as mybir
import concourse.tile as tile
from concourse.bass_utils import run_bass_kernel_spmd


def _register_ntff_hook():
    """If the image's antenv lacks axon_hooks, run_bass_kernel_spmd crashes
    under BASS_TRACE=1; synthesize the module from trn_boot's ctypes hook
    builder (silent no-op when unavailable)."""
    try:
        import antenv.axon_hooks  # noqa: F401
        return
    except ImportError:
        pass
    try:
        from trn_agent_boot.trn_boot import _ntff_profile_via_ctypes

        hook = _ntff_profile_via_ctypes("/opt/axon/libaxon_pjrt.so")
    except Exception:
        hook = None
    mod = types.ModuleType("antenv.axon_hooks")
    mod.get_axon_ntff_profile_hook = lambda: hook
    mod.set_axon_ntff_profile_hook = lambda h: None
    sys.modules["antenv.axon_hooks"] = mod


_register_ntff_hook()

B, S, V, D = 1024, 200, 100000, 128
N_CORES = 8
P = 128

_CACHE: dict = {}


def _chunk_sizes(G):
    """DMA chunking in g-tiles: tiny first chunk so matmul 0 starts early,
    ~5-tile steady state, small tail so the last matmuls aren't waiting on
    a big transfer."""
    steady = 5
    if G <= 2:
        return [1] * G
    sizes = [1]
    rem = G - 1
    while rem > steady + 2:
        sizes.append(steady)
        rem -= steady
    if rem > 3:
        sizes.append(rem - 3)
        rem = 3
    while rem:
        c = min(2, rem)
        sizes.append(c)
        rem -= c
    assert sum(sizes) == G
    return sizes


def _build_nc(G):
    nc = bacc.Bacc(
        "TRN2",
        target_bir_lowering=False,
        debug=False,
        num_devices=N_CORES,
    )
    f32 = mybir.dt.float32
    bf16 = mybir.dt.bfloat16
    fp8 = mybir.dt.float8e4

    cnt = nc.dram_tensor("cnt", [P, G * B], fp8, kind="ExternalInput")
    wsh = nc.dram_tensor("wsh", [P, G * D], bf16, kind="ExternalInput")
    out_t = nc.dram_tensor("out_t", [P, B], f32, kind="ExternalOutput")

    with tile.TileContext(nc) as tc:
        with (
            tc.tile_pool(name="const", bufs=1) as cpool,
            tc.tile_pool(name="psum", bufs=1, space="PSUM") as ppool,
        ):
            cnt_sb = cpool.tile([P, G * B], fp8)
            w_sb = cpool.tile([P, G * D], bf16)
            out_sb = cpool.tile([P, B], f32)
            warm_sb = cpool.tile([P, 512], bf16)

            # HAM warm-up sized to the idle window between body start and
            # chunk-0 arrival (~3.5 us): the PE cold clock is 1.2 GHz and
            # unthrottles after ~3.4 us of sustained activity, so these
            # dummy matmuls cost nothing and the real ones start warm.
            # (A 14-matmul chain was a net loss when it overran the window.)
            nc.vector.memset(warm_sb[:], 0.0)
            pwarm = ppool.tile([P, 512], f32, tag="pwarm")
            for k in range(8):
                nc.tensor.matmul(
                    pwarm[:],
                    warm_sb[:, 0:128],
                    warm_sb[:],
                    start=(k == 0),
                    stop=(k == 7),
                )

            # chunked input streams, counts/W pairwise interleaved so the
            # g-th matmul's operands land together (Tile adds per-chunk
            # deps).  (Scalar-queue W dispatch was tried and removed: it
            # starved the counts stream.)
            k = 0
            for sz in _chunk_sizes(G):
                hi = k + sz
                nc.sync.dma_start(
                    out=cnt_sb[:, k * B : hi * B], in_=cnt[:, k * B : hi * B]
                )
                nc.sync.dma_start(
                    out=w_sb[:, k * D : hi * D], in_=wsh[:, k * D : hi * D]
                )
                k = hi

            ps0 = ppool.tile([P, 512], f32, tag="ps0")
            ps1 = ppool.tile([P, 512], f32, tag="ps1")

            def mm(ps, g, lo, start, stop):
                nc.tensor.matmul(
                    ps[:],
                    w_sb[:, g * D : (g + 1) * D],
                    cnt_sb[:, g * B + lo : g * B + lo + 512],
                    start=start,
                    stop=stop,
                )

            # interleave the two halves until the last few tiles, then run
            # ps1's remainder first so its PSUM drain + out DMA overlap
            # ps0's final matmuls (staggered tail)
            STAG = min(5, G - 1)
            for g in range(G - STAG):
                mm(ps0, g, 0, g == 0, False)
                mm(ps1, g, 512, g == 0, False)
            for g in range(G - STAG, G):
                mm(ps1, g, 512, False, g == G - 1)
            nc.vector.tensor_copy(out=out_sb[:, 512:B], in_=ps1[:])
            nc.sync.dma_start(out=out_t[:, 512:B], in_=out_sb[:, 512:B])
            for g in range(G - STAG, G):
                mm(ps0, g, 0, False, g == G - 1)
            nc.vector.tensor_copy(out=out_sb[:, 0:512], in_=ps0[:])
            nc.sync.dma_start(out=out_t[:, 0:512], in_=out_sb[:, 0:512])

    nc.compile()
    return nc


def _get_nc(G=86):
    key = ("nc", G)
    if key not in _CACHE:
        _CACHE[key] = _build_nc(G)
    return _CACHE[key]


def _shard_layout(arr2d, ncols):
    """[G*128, ncols] -> [128, G*ncols] partition-major: out[p, g*ncols+j]
    = arr2d[g*128 + p, j]."""
    g = arr2d.shape[0] // P
    a = arr2d.reshape(g, P, ncols).transpose(1, 0, 2).reshape(P, g * ncols)
    return np.ascontiguousarray(a)


def _in_maps(token_ids, W, b):
    """Returns (in_maps, G)."""
    counts = np.zeros((B, V), dtype=np.int16)
    rows = np.repeat(np.arange(B, dtype=np.int64), S)
    np.add.at(counts, (rows, token_ids.ravel().astype(np.int64)), 1)
    if counts.max() > 16:
        raise ValueError("count > 16 not exact in fp8 e4m3")

    # active-vocab compaction: ship only rows some batch row references
    active = np.flatnonzero(counts.any(axis=0))
    M = active.size
    per = -(-M // N_CORES)          # rows per core
    G = max(1, -(-per // P))        # 128-row tiles per core
    VP = G * P

    Wb = W.astype(ml_dtypes.bfloat16)
    in_maps = []
    for c in range(N_CORES):
        idx = active[c * per : (c + 1) * per]
        csh = np.zeros((VP, B), dtype=ml_dtypes.float8_e4m3)
        csh[: idx.size] = counts[:, idx].T.astype(ml_dtypes.float8_e4m3)
        wshard = np.zeros((VP, D), dtype=ml_dtypes.bfloat16)
        wshard[: idx.size] = Wb[idx]
        in_maps.append(
            {"cnt": _shard_layout(csh, B), "wsh": _shard_layout(wshard, D)}
        )
    return in_maps, G


def _kernel_numpy(token_ids, W, b):
    out = np.tile(b.astype(np.float32), (B, 1))
    for i in range(B):
        out[i] += W[token_ids[i]].sum(axis=0)
    return out.astype(np.float32)


def kernel(token_ids, W, b, **kwargs):
    token_ids = np.ascontiguousarray(np.asarray(token_ids, dtype=np.int32))
    W = np.ascontiguousarray(np.asarray(W, dtype=np.float32))
    b = np.ascontiguousarray(np.asarray(b, dtype=np.float32))
    assert token_ids.shape == (B, S) and W.shape == (V, D) and b.shape == (D,)

    try:
        in_maps, G = _in_maps(token_ids, W, b)
    except ValueError:
        return _kernel_numpy(token_ids, W, b)

    nc = _get_nc(G)
    res = run_bass_kernel_spmd(nc, in_maps, core_ids=list(range(N_CORES)))
    acc = np.zeros((P, B), dtype=np.float32)
    for c in range(N_CORES):
        acc += np.asarray(res.results[c]["out_t"], dtype=np.float32)
    return (acc.T + b[None, :]).astype(np.float32)


# revision 25
# speedup vs baseline: 1.0753x; 1.0086x over previous
"""CountVectorizer Trainium2 kernel (v4: vocab-sharded counts matmul,
active-vocab compaction).

Computes out = counts @ W + b  where counts[b, v] = #{s: token_ids[b, s] == v}.

v2 (embedding-bag dma_gather) was SWDGE descriptor-generation bound:
~7.85 ns/gathered-row on the Q7 => ~282 us serial GpSimd (358 us total).
v3+ uses the dense formulation from the sharding hint: the vocab is sharded
across the 8 cores; each core streams its W shard (bf16) and a host-built
counts shard (fp8 e4m3 -- counts are small ints, exact in e4m3) and runs
   out_c[d, b] = sum_v W[v, d] * counts[v, b]
as accumulating PE matmuls: lhsT = W tile [128v, 128d] bf16 stationary,
rhs = counts tile [128v, 512b] fp8 moving, PSUM f32 (512-col halves --
a matmul cannot cross a PSUM bank).  The host sums the 8 partials and adds
the bias in f32, so the only error source is the bf16 W cast (~1.6e-3 rel,
gate 2e-2).

v4 compacts the vocab first: rows of counts that are all-zero across the
batch (P = (1-1/V)^(B*S) ~ 13%) are pruned on the host, and only active
W/counts rows ship.  Both HBM streams and the PE contraction shrink ~13%.

Per-core after compaction: ~2.8 MB W + ~11.2 MB counts (~39 us at
358 GB/s HBM); PE: ~86 tiles x 1024 cols ~ 40 us warm at 2.4 GHz.  DMAs
are chunked (counts first, small leading chunk) and the matmuls chase the
chunks, overlapping the two almost fully.  Both streams and the PE are at
their rooflines simultaneously.

Measured (local harness, core-0 NTFF): 57.4-61 us vs 358 us for the v2
gather baseline (~6.1x).  Head ~10 us (NEFF preamble + chunk-0 flight)
and tail ~6 us (PSUM drain + out DMA + final barrier) are fixed costs.
"""

import sys
import types

import numpy as np
import ml_dtypes

import concourse.bacc as bacc
import concourse.mybir as mybir
import concourse.tile as tile
from concourse.bass_utils import run_bass_kernel_spmd


def _register_ntff_hook():
    """If the image's antenv lacks axon_hooks, run_bass_kernel_spmd crashes
    under BASS_TRACE=1; synthesize the module from trn_boot's ctypes hook
    builder (silent no-op when unavailable)."""
    try:
        import antenv.axon_hooks  # noqa: F401
        return
    except ImportError:
        pass
    try:
        from trn_agent_boot.trn_boot import _ntff_profile_via_ctypes

        hook = _ntff_profile_via_ctypes("/opt/axon/libaxon_pjrt.so")
    except Exception:
        hook = None
    mod = types.ModuleType("antenv.axon_hooks")
    mod.get_axon_ntff_profile_hook = lambda: hook
    mod.set_axon_ntff_profile_hook = lambda h: None
    sys.modules["antenv.axon_hooks"] = mod


_register_ntff_hook()

B, S, V, D = 1024, 200, 100000, 128
N_CORES = 8
P = 128

_CACHE: dict = {}


def _chunk_sizes(G):
    """DMA chunking in g-tiles: tiny first chunk so matmul 0 starts early,
    ~5-tile steady state, small tail so the last matmuls aren't waiting on
    a big transfer."""
    steady = 5
    if G <= 2:
        return [1] * G
    sizes = [1]
    rem = G - 1
    while rem > steady + 2:
        sizes.append(steady)
        rem -= steady
    if rem > 3:
        sizes.append(rem - 3)
        rem = 3
    while rem:
        c = min(2, rem)
        sizes.append(c)
        rem -= c
    assert sum(sizes) == G
    return sizes


def _build_nc(G):
    nc = bacc.Bacc(
        "TRN2",
        target_bir_lowering=False,
        debug=False,
        num_devices=N_CORES,
    )
    f32 = mybir.dt.float32
    bf16 = mybir.dt.bfloat16
    fp8 = mybir.dt.float8e4

    cnt = nc.dram_tensor("cnt", [P, G * B], fp8, kind="ExternalInput")
    wsh = nc.dram_tensor("wsh", [P, G * D], bf16, kind="ExternalInput")
    out_t = nc.dram_tensor("out_t", [P, B], f32, kind="ExternalOutput")

    with tile.TileContext(nc) as tc:
        with (
            tc.tile_pool(name="const", bufs=1) as cpool,
            tc.tile_pool(name="psum", bufs=1, space="PSUM") as ppool,
        ):
            cnt_sb = cpool.tile([P, G * B], fp8)
            w_sb = cpool.tile([P, G * D], bf16)
            out_sb = cpool.tile([P, B], f32)
            warm_sb = cpool.tile([P, 512], bf16)

            # HAM warm-up sized to the idle window between body start and
            # chunk-0 arrival (~3.5 us): the PE cold clock is 1.2 GHz and
            # unthrottles after ~3.4 us of sustained activity, so these
            # dummy matmuls cost nothing and the real ones start warm.
            # (A 14-matmul chain was a net loss when it overran the window.)
            nc.vector.memset(warm_sb[:], 0.0)
            pwarm = ppool.tile([P, 512], f32, tag="pwarm")
            for k in range(8):
                nc.tensor.matmul(
                    pwarm[:],
                    warm_sb[:, 0:128],
                    warm_sb[:],
                    start=(k == 0),
                    stop=(k == 7),
                )

            # chunked input streams, counts/W pairwise interleaved so the
            # g-th matmul's operands land together (Tile adds per-chunk
            # deps).  (Scalar-queue W dispatch was tried and removed: it
            # starved the counts stream.)
            k = 0
            for sz in _chunk_sizes(G):
                hi = k + sz
                nc.sync.dma_start(
                    out=cnt_sb[:, k * B : hi * B], in_=cnt[:, k * B : hi * B]
                )
                nc.sync.dma_start(
                    out=w_sb[:, k * D : hi * D], in_=wsh[:, k * D : hi * D]
                )
                k = hi

            ps0 = ppool.tile([P, 512], f32, tag="ps0")
            ps1 = ppool.tile([P, 512], f32, tag="ps1")

            def mm(ps, g, lo, start, stop):
                nc.tensor.matmul(
                    ps[:],
                    w_sb[:, g * D : (g + 1) * D],
                    cnt_sb[:, g * B + lo : g * B + lo + 512],
                    start=start,
                    stop=stop,
                )

            # interleave the two halves until the last few tiles, then run
            # ps1's remainder first so its PSUM drain + out DMA overlap
            # ps0's final matmuls (staggered tail)
            STAG = min(10, G - 1)
            for g in range(G - STAG):
                mm(ps0, g, 0, g == 0, False)
                mm(ps1, g, 512, g == 0, False)
            for g in range(G - STAG, G):
                mm(ps1, g, 512, False, g == G - 1)
            nc.vector.tensor_copy(out=out_sb[:, 512:B], in_=ps1[:])
            nc.sync.dma_start(out=out_t[:, 512:B], in_=out_sb[:, 512:B])
            for g in range(G - STAG, G):
                mm(ps0, g, 0, False, g == G - 1)
            nc.vector.tensor_copy(out=out_sb[:, 0:512], in_=ps0[:])
            nc.sync.dma_start(out=out_t[:, 0:512], in_=out_sb[:, 0:512])

    nc.compile()
    return nc


def _get_nc(G=86):
    key = ("nc", G)
    if key not in _CACHE:
        _CACHE[key] = _build_nc(G)
    return _CACHE[key]


def _shard_layout(arr2d, ncols):
    """[G*128, ncols] -> [128, G*ncols] partition-major: out[p, g*ncols+j]
    = arr2d[g*128 + p, j]."""
    g = arr2d.shape[0] // P
    a = arr2d.reshape(g, P, ncols).transpose(1, 0, 2).reshape(P, g * ncols)
    return np.ascontiguousarray(a)


def _in_maps(token_ids, W, b):
    """Returns (in_maps, G)."""
    counts = np.zeros((B, V), dtype=np.int16)
    rows = np.repeat(np.arange(B, dtype=np.int64), S)
    np.add.at(counts, (rows, token_ids.ravel().astype(np.int64)), 1)
    if counts.max() > 16:
        raise ValueError("count > 16 not exact in fp8 e4m3")

    # active-vocab compaction: ship only rows some batch row references
    active = np.flatnonzero(counts.any(axis=0))
    M = active.size
    per = -(-M // N_CORES)          # rows per core
    G = max(1, -(-per // P))        # 128-row tiles per core
    VP = G * P

    Wb = W.astype(ml_dtypes.bfloat16)
    in_maps = []
    for c in range(N_CORES):
        idx = active[c * per : (c + 1) * per]
        csh = np.zeros((VP, B), dtype=ml_dtypes.float8_e4m3)
        csh[: idx.size] = counts[:, idx].T.astype(ml_dtypes.float8_e4m3)
        wshard = np.zeros((VP, D), dtype=ml_dtypes.bfloat16)
        wshard[: idx.size] = Wb[idx]
        in_maps.append(
            {"cnt": _shard_layout(csh, B), "wsh": _shard_layout(wshard, D)}
        )
    return in_maps, G


def _kernel_numpy(token_ids, W, b):
    out = np.tile(b.astype(np.float32), (B, 1))
    for i in range(B):
        out[i] += W[token_ids[i]].sum(axis=0)
    return out.astype(np.float32)


def kernel(token_ids, W, b, **kwargs):
    token_ids = np.ascontiguousarray(np.asarray(token_ids, dtype=np.int32))
    W = np.ascontiguousarray(np.asarray(W, dtype=np.float32))
    b = np.ascontiguousarray(np.asarray(b, dtype=np.float32))
    assert token_ids.shape == (B, S) and W.shape == (V, D) and b.shape == (D,)

    try:
        in_maps, G = _in_maps(token_ids, W, b)
    except ValueError:
        return _kernel_numpy(token_ids, W, b)

    nc = _get_nc(G)
    res = run_bass_kernel_spmd(nc, in_maps, core_ids=list(range(N_CORES)))
    acc = np.zeros((P, B), dtype=np.float32)
    for c in range(N_CORES):
        acc += np.asarray(res.results[c]["out_t"], dtype=np.float32)
    return (acc.T + b[None, :]).astype(np.float32)
